# revision 11
# baseline (speedup 1.0000x reference)
"""Trainium2 Bass kernel for 0.7*BCEWithLogits + 0.3*MultiLabelMarginLoss.

Math (per row of N = B*T rows, V = 128 classes; output = mean over rows):
  bce_row = mean_n[ softplus(x_n) - x_n*t_n ]
          = (1/V) [ sum_{neg} softplus(x_n) + sum_{pos} softplus(-x_p) ]
  mlm_row = (1/V) sum_{p in pos} sum_{n in neg} relu(1 - x_p + x_n)

Sharding: host sorts rows by positive count, deals them round-robin to the
8 cores (identical schedule per core -> one NEFF), packs each core's 16
row-blocks side by side as u [128, 16*128] bf16 with the positives masked
to -30, plus two threshold tables holding the NEGATED positive logits
t' = -x_p (pads -1e9): tb_dup [128, 16*32] with every slot value twice
(for the DVE hinge op's 2x word-latch), and tb_1x [128, 16*16] (for the
ACT softplus pass).

Device math per core:
  BCE:   one Exp pass + one Ln(1+e) pass with accum over [tb_1x | u]
         (softplus(t') = softplus(-x_p); pads/masks underflow to 0).
  Hinge: one custom DVE instruction per block running a hand-written
         3-state uop program (seed / latch / steady) in 2X_1PORT perf
         mode: the per-subdim threshold t' is latched from src1 (+1.0
         folded in via the ONE_F32 input lane), and the steady state
         computes relu(u + t' + 1) for TWO bf16 elements per cycle,
         keeping a running sum in block 7's out-flop which it streams to
         the dst; the LAST dst element of each block is the block's
         hinge total (this also holds if the engine falls back to 1x).
         Block z-slots are end-aligned at uniform stride in one arena so
         a single strided ACT copy gathers the 16 block sums into the
         accumulator tile.

Blocks are processed heaviest-first so the tail of the DVE stream is
cheap; inputs ship as 3 parallel DMAs (2 on the SP queue, 1 on the ACT
queue) so descriptor generation overlaps. The device ships acc [128, 17]
(16 hinge sums + 1 softplus sum); the host applies the 0.7/0.3 weights
and the mean. All arithmetic is on device; the host only permutes /
gathers / constant-fills input values and sums the 8 core partials.
"""

import sys

sys.path.insert(0, "/opt/trn_rl_repo")

import numpy as np
import ml_dtypes

import concourse.bacc as bacc
import concourse.tile as tile
from concourse import mybir
from concourse.bass_utils import run_bass_kernel_spmd

F32 = mybir.dt.float32
BF16 = mybir.dt.bfloat16
ACTF = mybir.ActivationFunctionType

B, T, V = 16, 1024, 128
ROWS = B * T
N_CORES = 8
RPC = ROWS // N_CORES             # 2048 rows per core
P = 128                           # rows per block
NBLK = RPC // P                   # 16 blocks
SLOTS = 16                        # positive-table slots per block
SMAX = 16                         # z-arena slot size in subdims (>= max npos)
ZSLOT = SMAX * V                  # z-arena columns per block

D_DUP = NBLK * 2 * SLOTS          # 512: duplicated threshold table cols
D_1X = NBLK * SLOTS               # 256: plain threshold table cols
XCOLS = D_DUP + D_1X + NBLK * V   # 2816 total input cols (bf16)
# xall layout: [ tb_dup | u_blk0 | u_blk1 | tb_1x | u_blk2..15 ] so the
# first DMA chunk carries exactly what the two heaviest hinges need.
U0A = D_DUP                       # u blocks 0-1 start
T1X = D_DUP + 2 * V               # plain table start
U2 = T1X + D_1X                   # u blocks 2.. start


def _ucol(j):
    return U0A + j * V if j < 2 else U2 + (j - 2) * V

PADV = -1.0e9                     # table pad (relu and softplus neutral)
NEGV = -30.0                      # masked-positive value in u
BCE_W = 0.7
MLM_W = 0.3


# --------------------------------------------------------------------------
# custom DVE op: 3-state subdim hinge with hand-written 1x and 2x programs
# --------------------------------------------------------------------------

def _build_hinge_uops(two_x: bool):
    """seed / latch / steady.

    Input lanes (chains): d0=SRC_0, d1=SRC_1, d2=ZERO, d3=SRC_0_HI (2x),
    d4=ONE_F32 (latch only; steady reuses d4/d5 internally).
    The latch computes t'+1 and captures it in blk1's swap flop (same-cycle
    combinational path), consuming one src1 element (1x, repeat=2 over the
    duplicated table) or one 2-element word (2x, repeat=1).
    The steady keeps the running sum in blk7's out-flop and streams it to
    the dst (WR0_LO at 1x, WR0_HI at 2x), so the last dst element is the
    block total.
    """
    from concourse.dve_uop import (
        AluInp, AluOp, DelayInp, InpSel, OutPath, OutSel, Trigger, UopConfig,
    )

    D = DelayInp
    A = AluInp

    def new_state(latch_lane=False):
        u = UopConfig()
        u.enable_input(InpSel.SRC_0, 1)       # chain d0
        u.enable_input(InpSel.SRC_1, 2)       # chain d1
        u.enable_input(InpSel.ZERO, 3)        # chain d2
        if two_x:
            u.enable_input(InpSel.SRC_0_HI, 4)  # chain d3
        if latch_lane:
            u.enable_input(InpSel.ONE_F32, 5)   # chain d4
        return u

    chains = (0, 1, 2, 3) if two_x else (0, 1, 2)

    # ---- state 0: SEED -- zero blk7's out-flop (the accumulator) ----
    seed = new_state()
    for b in range(7):
        seed.datapath_config[b].pass_through_delay(*chains)
    seed.datapath_config[7].enable_alu(AluOp.BYPASS, A.PREV_DELAY_2, A.PREV_DELAY_2)
    seed.repeat_count = 1
    seed.trigger = (Trigger.COUNT, Trigger.NONE, Trigger.NONE)
    seed.next_uop = (1, 0, 0)

    # ---- state 1: LATCH -- swap(blk1) <- t' + 1 ----
    lat = new_state(latch_lane=True)
    lc = chains + (4,)
    b0 = lat.datapath_config[0]
    b0.enable_alu(AluOp.ADD, A.PREV_DELAY_1, A.PREV_DELAY_4)   # t' + 1
    b0.pass_through_delay(*lc)
    b1 = lat.datapath_config[1]
    b1.op = AluOp.BYPASS
    b1.alu_src0 = A.PREV_ALU_OUT
    b1.alu_src1 = A.PREV_ALU_OUT
    b1.swap_enable = 1                                         # capture
    b1.pass_through_delay(*lc)
    for b in range(2, 7):
        lat.datapath_config[b].pass_through_delay(*lc)
    # blk7 untouched: preserves the running sum across subdim boundaries
    lat.require_inp1 = 1
    lat.repeat_count = 1 if two_x else 2
    lat.trigger = (Trigger.COUNT, Trigger.NONE, Trigger.NONE)
    lat.next_uop = (2, 0, 0)

    # ---- state 2: STEADY ----
    st = new_state()
    st.require_inp0 = 1
    dp = st.datapath_config
    if two_x:
        # x0 = S0 + (t'+1); relu0 = max(x0, 0)
        # x1 + relu0 = relu0 + x0 + S0H - S0; pairsum = max(x1+relu0, relu0)
        dp[0].pass_through_delay(0, 1, 2, 3)
        dp[1].enable_alu(AluOp.ADD, A.PREV_DELAY_0, A.CURR_SWAP_OUT)
        dp[1].pass_through_delay(0, 2, 3)
        dp[2].enable_alu(AluOp.MAX, A.PREV_ALU_OUT, A.PREV_DELAY_2)
        dp[2].enable_delay_from_src(D.PREV_ALU_OUT, 4)         # d4 <- x0
        dp[2].pass_through_delay(0, 3)
        dp[3].enable_alu(AluOp.ADD, A.PREV_ALU_OUT, A.PREV_DELAY_4)  # relu0+x0
        dp[3].enable_delay_from_src(D.PREV_ALU_OUT, 5)         # d5 <- relu0
        dp[3].pass_through_delay(0, 3)
        dp[4].enable_alu(AluOp.ADD, A.PREV_ALU_OUT, A.PREV_DELAY_3)  # + S0H
        dp[4].pass_through_delay(0, 5)
        dp[5].enable_alu(AluOp.SUBTRACT, A.PREV_ALU_OUT, A.PREV_DELAY_0)  # - S0
        dp[5].pass_through_delay(5)
        dp[6].enable_alu(AluOp.MAX, A.PREV_ALU_OUT, A.PREV_DELAY_5)  # pairsum
        dp[6].pass_through_delay(5)
        dp[7].enable_alu(AluOp.ADD, A.CURR_ALU_OUT, A.PREV_ALU_OUT)  # acc
        dp[7].pass_through_delay(5)
        st.enable_output(OutSel.DELAY_5, OutPath.WR0_LO)       # relu0
        st.enable_output(OutSel.ALU_OUT, OutPath.WR0_HI)       # running acc
    else:
        dp[0].pass_through_delay(0, 1, 2)
        dp[1].enable_alu(AluOp.ADD, A.PREV_DELAY_0, A.CURR_SWAP_OUT)
        dp[1].pass_through_delay(2)
        dp[2].enable_alu(AluOp.MAX, A.PREV_ALU_OUT, A.PREV_DELAY_2)  # relu
        for b in (3, 4, 5, 6):
            dp[b].enable_alu(AluOp.BYPASS, A.PREV_ALU_OUT, A.PREV_ALU_OUT)
        dp[7].enable_alu(AluOp.ADD, A.CURR_ALU_OUT, A.PREV_ALU_OUT)  # acc
        st.enable_output(OutSel.ALU_OUT, OutPath.WR0_LO)       # running acc
    st.trigger = (Trigger.SRC_TENSOR_DONE, Trigger.SUB_DIM_DONE, Trigger.NONE)
    st.next_uop = (0, 1, 0)
    return [seed, lat, st]


class _Hinge2xOp:
    """Duck-type of dve_ops.DveOp with hand-written uops (+2x variant).

    Semantics per instruction (subdim form):
      in0 [P, S, V] bf16 (u block), in1 [P, 2S] bf16 (duplicated t'),
      out [P, S*V] bf16; out[:, -1] = sum_{s,n} relu(in0[p,s,n] + t'[p,s] + 1).
    """

    name = "HINGE2X_ANT"
    subdim = True

    def __init__(self):
        from concourse.dve_spec import Spec, Src0, Src1, relu

        def _ref(in0, in1, c0, c1, c2):
            a = np.asarray(in0, np.float32)
            S = a.shape[1]
            thr = np.asarray(in1, np.float32)[:, : 2 * S : 2]
            z = np.maximum(a + thr[:, :, None] + 1.0, 0.0)
            acc = z.sum((1, 2))
            zf = z.reshape(z.shape[0], -1).copy()
            zf[:, -1] = acc
            return zf

        self.spec = Spec(body=relu(Src0 + Src1), reference=_ref)
        self._cache = {}

    def compile(self, ver):
        from concourse.dve_uop import DveOpSpec
        from concourse.dve_ops import get_dve_sub_opcode

        if ver not in self._cache:
            r = DveOpSpec(
                name=self.name,
                opcode=get_dve_sub_opcode(self.name),
                uops=_build_hinge_uops(False),
                uops_2x=_build_hinge_uops(True),
                perf_max=1,
                rd1_en=True,
            )
            r.validate(ver)
            self._cache[ver] = r
        return self._cache[ver]


def _register_op():
    from concourse import dve_ops as dops

    if hasattr(dops, "ANT_HINGE2X_OP"):
        return dops.ANT_HINGE2X_OP
    op = _Hinge2xOp()
    opc = max(dops._SUB_OPCODE_FOR_NAME.values()) + 1
    dops.OPS.append(op)
    dops.CUSTOM_DVE_SPECS[op.name] = op.spec
    dops._SUB_OPCODE_FOR_NAME[op.name] = opc
    dops.ANT_HINGE2X_OP = op
    return op


HINGE2X = _register_op()


def _act_set_id(nc, name):
    from concourse.hw_specs import get_activation_tables

    return list(get_activation_tables(nc.m.arch)).index(name)


# --------------------------------------------------------------------------
# per-core bass program
# --------------------------------------------------------------------------

def build_nc(schedule):
    """schedule: per-block hinge-slot counts in COMPUTE order (descending)."""
    nc = bacc.Bacc("TRN2", target_bir_lowering=False, debug=False)
    xg_dram = nc.dram_tensor("xg", [P, XCOLS], BF16, kind="ExternalInput")
    out_dram = nc.dram_tensor("out", [P, NBLK + 1], F32, kind="ExternalOutput")
    xg_ap = xg_dram.ap()

    # input DMA split: chunk boundaries in xall columns
    CUT1 = T1X                 # dup table + two heaviest blocks
    CUT2 = U2 + 4 * V          # plain table + blocks 2..5

    with tile.TileContext(nc) as tc:
        with (
            tc.tile_pool(name="const", bufs=1) as cpool,
            tc.tile_pool(name="work", bufs=1) as wpool,
        ):
            nc.scalar.add_instruction(
                mybir.InstLoadActFuncSet(
                    name=nc.get_next_instruction_name(), ins=[], outs=[],
                    act_func_set_id=_act_set_id(
                        nc, "natural_log_exp_and_others"
                    ),
                )
            )
            xall = cpool.tile([P, XCOLS], BF16, tag="xall")
            nc.sync.dma_start(xall[:, 0:CUT1], xg_ap[:, 0:CUT1])
            nc.gpsimd.dma_start(xall[:, CUT1:CUT2], xg_ap[:, CUT1:CUT2])
            nc.sync.dma_start(xall[:, CUT2:XCOLS], xg_ap[:, CUT2:XCOLS])

            zarena = wpool.tile([P, NBLK * ZSLOT], BF16, tag="zarena")
            earena = wpool.tile([P, D_1X + NBLK * V], F32, tag="earena")
            acc = cpool.tile([P, NBLK + 1], F32, tag="acc")

            # ---- hinge: one custom DVE instruction per block ----
            for j in range(NBLK):
                S = schedule[j]
                x_blk = xall[:, _ucol(j) : _ucol(j) + V]
                # in0 reads the block's V columns S times (stride-0 repeat)
                in0 = x_blk.unsqueeze(1).broadcast_to([P, S, V])
                t2 = xall[:, j * 2 * SLOTS : j * 2 * SLOTS + 2 * S]
                z0 = (j + 1) * ZSLOT - S * V
                z = zarena[:, z0 : (j + 1) * ZSLOT]
                inst = nc.vector._custom_dve(
                    HINGE2X,
                    out=z.rearrange("p (s n) -> p s n", s=S),
                    in0=in0,
                    in1=t2,
                    s0=0.0, s1=0.0,
                )
                inst.ins.perf_max = 1

            # ---- BCE: exp then ln(1+e) with accum over [u0 | tb_1x | u1..] ----
            e_cuts = [D_DUP, CUT1, CUT2, XCOLS]
            for a, b in zip(e_cuts[:-1], e_cuts[1:]):
                nc.scalar.activation(
                    earena[:, a - D_DUP : b - D_DUP], xall[:, a:b],
                    ACTF.Exp, bias=0.0, scale=1.0,
                )
            lns = wpool.tile([P, D_1X + NBLK * V], F32, tag="lns")
            nc.scalar.activation(
                lns[:], earena[:], ACTF.Ln, bias=1.0, scale=1.0,
                accum_out=acc[:, NBLK : NBLK + 1],
            )

            # ---- gather the 16 block sums (last element of each z slot);
            # split so the bulk gather overlaps the tail hinge blocks ----
            gap = zarena[:].rearrange("p (b z) -> p b z", b=NBLK)[
                :, :, ZSLOT - 1 : ZSLOT
            ].squeeze(2)
            GCUT = 12
            nc.scalar.activation(
                acc[:, 0:GCUT], gap[:, 0:GCUT], ACTF.Identity,
                bias=0.0, scale=1.0,
            )
            nc.scalar.activation(
                acc[:, GCUT:NBLK], gap[:, GCUT:NBLK], ACTF.Identity,
                bias=0.0, scale=1.0,
            )

            nc.sync.dma_start(out_dram.ap()[:, :], acc[:])

    nc.compile()
    return nc


_NC_CACHE = {}


def _get_nc(schedule):
    if schedule not in _NC_CACHE:
        _NC_CACHE[schedule] = build_nc(schedule)
    return _NC_CACHE[schedule]


def _pack_blocks(a, order):
    """[RPC, W] row-major -> [P, NBLK*W], blocks side by side in `order`."""
    w = a.shape[1]
    return np.ascontiguousarray(
        a.reshape(NBLK, P, w)[order].transpose(1, 0, 2).reshape(P, NBLK * w)
    )


def _shard(x, t):
    """npos-sorted round-robin shard, heaviest block first.

    Returns (schedule, in_maps); schedule is in compute (descending) order.
    """
    npos = (t > 0.5).sum(axis=1)
    assert npos.max() <= SLOTS, f"row with {npos.max()} positives > {SLOTS}"
    order = np.argsort(npos, kind="stable")
    ns = npos[order]
    sched_asc = [
        max(1, int(ns[(b + 1) * (N_CORES * P) - 1])) for b in range(NBLK)
    ]
    blk_order = list(range(NBLK - 1, -1, -1))     # descending slot counts
    schedule = tuple(sched_asc[b] for b in blk_order)
    xs = x[order]
    ps = t[order] > 0.5
    nsr = ns
    in_maps = []
    for c in range(N_CORES):
        xc = xs[c::N_CORES]                       # [RPC, V]
        pc = ps[c::N_CORES]
        nc_ = nsr[c::N_CORES]
        uc = np.where(pc, np.float32(NEGV), xc).astype(ml_dtypes.bfloat16)
        colorder = np.argsort(~pc, axis=1, kind="stable")[:, :SLOTS]
        vals = np.take_along_axis(xc, colorder, axis=1)
        mask = np.arange(SLOTS)[None, :] < nc_[:, None]
        tbl = np.where(mask, -vals, np.float32(PADV)).astype(ml_dtypes.bfloat16)
        tbl_dup = np.repeat(tbl, 2, axis=1)       # [RPC, 32]
        u_pack = _pack_blocks(uc, blk_order)
        in_maps.append(
            {
                "xg": np.concatenate(
                    [
                        _pack_blocks(tbl_dup, blk_order),
                        u_pack[:, 0 : 2 * V],
                        _pack_blocks(tbl, blk_order),
                        u_pack[:, 2 * V :],
                    ],
                    axis=1,
                )
            }
        )
    return schedule, in_maps


def _reduce_core(out_arr):
    """Device partials [P, NBLK+1] -> unnormalized core total
    V * sum_rows(0.7*bce_row + 0.3*mlm_row)."""
    a = out_arr.astype(np.float64)
    hinge = a[:, 0:NBLK].sum()
    sp = a[:, NBLK].sum()
    return BCE_W * sp + MLM_W * hinge


def kernel(logits: np.ndarray, targets: np.ndarray) -> np.ndarray:
    x = np.asarray(logits, dtype=np.float32).reshape(ROWS, V)
    t = np.asarray(targets, dtype=np.float32).reshape(ROWS, V)
    schedule, in_maps = _shard(x, t)
    nc = _get_nc(schedule)
    res = run_bass_kernel_spmd(nc, in_maps, list(range(N_CORES)))
    total = sum(
        _reduce_core(res.results[c]["out"]) for c in range(N_CORES)
    )
    return np.float32(total / V / ROWS)


# revision 12
# speedup vs baseline: 1.0213x; 1.0213x over previous
"""Trainium2 Bass kernel for 0.7*BCEWithLogits + 0.3*MultiLabelMarginLoss.

Math (per row of N = B*T rows, V = 128 classes; output = mean over rows):
  bce_row = mean_n[ softplus(x_n) - x_n*t_n ]
          = (1/V) [ sum_{neg} softplus(x_n) + sum_{pos} softplus(-x_p) ]
  mlm_row = (1/V) sum_{p in pos} sum_{n in neg} relu(1 - x_p + x_n)

Sharding: host sorts rows by positive count, deals them round-robin to the
8 cores (identical schedule per core -> one NEFF), packs each core's 16
row-blocks side by side as u [128, 16*128] bf16 with the positives masked
to -30, plus two threshold tables holding the NEGATED positive logits
t' = -x_p (pads -1e9): tb_dup [128, 16*32] with every slot value twice
(for the DVE hinge op's 2x word-latch), and tb_1x [128, 16*16] (for the
ACT softplus pass).

Device math per core:
  BCE:   one Exp pass + one Ln(1+e) pass with accum over [tb_1x | u]
         (softplus(t') = softplus(-x_p); pads/masks underflow to 0).
  Hinge: one custom DVE instruction per block running a hand-written
         3-state uop program (seed / latch / steady) in 2X_1PORT perf
         mode: the per-subdim threshold t' is latched from src1 (+1.0
         folded in via the ONE_F32 input lane), and the steady state
         computes relu(u + t' + 1) for TWO bf16 elements per cycle,
         keeping a running sum in block 7's out-flop which it streams to
         the dst; the LAST dst element of each block is the block's
         hinge total (this also holds if the engine falls back to 1x).
         Block z-slots are end-aligned at uniform stride in one arena so
         a single strided ACT copy gathers the 16 block sums into the
         accumulator tile.

Blocks are processed heaviest-first so the tail of the DVE stream is
cheap; inputs ship as 3 parallel DMAs (2 on the SP queue, 1 on the ACT
queue) so descriptor generation overlaps. The device ships acc [128, 17]
(16 hinge sums + 1 softplus sum); the host applies the 0.7/0.3 weights
and the mean. All arithmetic is on device; the host only permutes /
gathers / constant-fills input values and sums the 8 core partials.
"""

import sys

sys.path.insert(0, "/opt/trn_rl_repo")

import numpy as np
import ml_dtypes

import concourse.bacc as bacc
import concourse.tile as tile
from concourse import mybir
from concourse.bass_utils import run_bass_kernel_spmd

F32 = mybir.dt.float32
BF16 = mybir.dt.bfloat16
ACTF = mybir.ActivationFunctionType

B, T, V = 16, 1024, 128
ROWS = B * T
N_CORES = 8
RPC = ROWS // N_CORES             # 2048 rows per core
P = 128                           # rows per block
NBLK = RPC // P                   # 16 blocks
SLOTS = 16                        # positive-table slots per block
SMAX = 16                         # z-arena slot size in subdims (>= max npos)
ZSLOT = SMAX * V                  # z-arena columns per block

D_DUP = NBLK * 2 * SLOTS          # 512: duplicated threshold table cols
D_1X = NBLK * SLOTS               # 256: plain threshold table cols
XCOLS = D_DUP + D_1X + NBLK * V   # 2816 total input cols (bf16)
# xall layout: [ tb_dup | u_blk0 | u_blk1 | tb_1x | u_blk2..15 ] so the
# first DMA chunk carries exactly what the two heaviest hinges need.
U0A = D_DUP                       # u blocks 0-1 start
T1X = D_DUP + 2 * V               # plain table start
U2 = T1X + D_1X                   # u blocks 2.. start


def _ucol(j):
    return U0A + j * V if j < 2 else U2 + (j - 2) * V

PADV = -1.0e9                     # table pad (relu and softplus neutral)
NEGV = -30.0                      # masked-positive value in u
BCE_W = 0.7
MLM_W = 0.3


# --------------------------------------------------------------------------
# custom DVE op: 3-state subdim hinge with hand-written 1x and 2x programs
# --------------------------------------------------------------------------

def _build_hinge_uops(two_x: bool):
    """seed / latch / steady.

    Input lanes (chains): d0=SRC_0, d1=SRC_1, d2=ZERO, d3=SRC_0_HI (2x),
    d4=ONE_F32 (latch only; steady reuses d4/d5 internally).
    The latch computes t'+1 and captures it in blk1's swap flop (same-cycle
    combinational path), consuming one src1 element (1x, repeat=2 over the
    duplicated table) or one 2-element word (2x, repeat=1).
    The steady keeps the running sum in blk7's out-flop and streams it to
    the dst (WR0_LO at 1x, WR0_HI at 2x), so the last dst element is the
    block total.
    """
    from concourse.dve_uop import (
        AluInp, AluOp, DelayInp, InpSel, OutPath, OutSel, Trigger, UopConfig,
    )

    D = DelayInp
    A = AluInp

    def new_state(latch_lane=False):
        u = UopConfig()
        u.enable_input(InpSel.SRC_0, 1)       # chain d0
        u.enable_input(InpSel.SRC_1, 2)       # chain d1
        u.enable_input(InpSel.ZERO, 3)        # chain d2
        if two_x:
            u.enable_input(InpSel.SRC_0_HI, 4)  # chain d3
        if latch_lane:
            u.enable_input(InpSel.ONE_F32, 5)   # chain d4
        return u

    chains = (0, 1, 2, 3) if two_x else (0, 1, 2)

    # ---- state 0: SEED -- zero blk7's out-flop (the accumulator) ----
    seed = new_state()
    for b in range(7):
        seed.datapath_config[b].pass_through_delay(*chains)
    seed.datapath_config[7].enable_alu(AluOp.BYPASS, A.PREV_DELAY_2, A.PREV_DELAY_2)
    seed.repeat_count = 1
    seed.trigger = (Trigger.COUNT, Trigger.NONE, Trigger.NONE)
    seed.next_uop = (1, 0, 0)

    # ---- state 1: LATCH -- swap(blk1) <- t' + 1 ----
    lat = new_state(latch_lane=True)
    lc = chains + (4,)
    b0 = lat.datapath_config[0]
    b0.enable_alu(AluOp.ADD, A.PREV_DELAY_1, A.PREV_DELAY_4)   # t' + 1
    b0.pass_through_delay(*lc)
    b1 = lat.datapath_config[1]
    b1.op = AluOp.BYPASS
    b1.alu_src0 = A.PREV_ALU_OUT
    b1.alu_src1 = A.PREV_ALU_OUT
    b1.swap_enable = 1                                         # capture
    b1.pass_through_delay(*lc)
    for b in range(2, 7):
        lat.datapath_config[b].pass_through_delay(*lc)
    # blk7 untouched: preserves the running sum across subdim boundaries
    lat.require_inp1 = 1
    lat.repeat_count = 1 if two_x else 2
    lat.trigger = (Trigger.COUNT, Trigger.NONE, Trigger.NONE)
    lat.next_uop = (2, 0, 0)

    # ---- state 2: STEADY ----
    st = new_state()
    st.require_inp0 = 1
    dp = st.datapath_config
    if two_x:
        # x0 = S0 + (t'+1); relu0 = max(x0, 0)
        # x1 + relu0 = relu0 + x0 + S0H - S0; pairsum = max(x1+relu0, relu0)
        dp[0].pass_through_delay(0, 1, 2, 3)
        dp[1].enable_alu(AluOp.ADD, A.PREV_DELAY_0, A.CURR_SWAP_OUT)
        dp[1].pass_through_delay(0, 2, 3)
        dp[2].enable_alu(AluOp.MAX, A.PREV_ALU_OUT, A.PREV_DELAY_2)
        dp[2].enable_delay_from_src(D.PREV_ALU_OUT, 4)         # d4 <- x0
        dp[2].pass_through_delay(0, 3)
        dp[3].enable_alu(AluOp.ADD, A.PREV_ALU_OUT, A.PREV_DELAY_4)  # relu0+x0
        dp[3].enable_delay_from_src(D.PREV_ALU_OUT, 5)         # d5 <- relu0
        dp[3].pass_through_delay(0, 3)
        dp[4].enable_alu(AluOp.ADD, A.PREV_ALU_OUT, A.PREV_DELAY_3)  # + S0H
        dp[4].pass_through_delay(0, 5)
        dp[5].enable_alu(AluOp.SUBTRACT, A.PREV_ALU_OUT, A.PREV_DELAY_0)  # - S0
        dp[5].pass_through_delay(5)
        dp[6].enable_alu(AluOp.MAX, A.PREV_ALU_OUT, A.PREV_DELAY_5)  # pairsum
        dp[6].pass_through_delay(5)
        dp[7].enable_alu(AluOp.ADD, A.CURR_ALU_OUT, A.PREV_ALU_OUT)  # acc
        dp[7].pass_through_delay(5)
        st.enable_output(OutSel.DELAY_5, OutPath.WR0_LO)       # relu0
        st.enable_output(OutSel.ALU_OUT, OutPath.WR0_HI)       # running acc
    else:
        dp[0].pass_through_delay(0, 1, 2)
        dp[1].enable_alu(AluOp.ADD, A.PREV_DELAY_0, A.CURR_SWAP_OUT)
        dp[1].pass_through_delay(2)
        dp[2].enable_alu(AluOp.MAX, A.PREV_ALU_OUT, A.PREV_DELAY_2)  # relu
        for b in (3, 4, 5, 6):
            dp[b].enable_alu(AluOp.BYPASS, A.PREV_ALU_OUT, A.PREV_ALU_OUT)
        dp[7].enable_alu(AluOp.ADD, A.CURR_ALU_OUT, A.PREV_ALU_OUT)  # acc
        st.enable_output(OutSel.ALU_OUT, OutPath.WR0_LO)       # running acc
    st.trigger = (Trigger.SRC_TENSOR_DONE, Trigger.SUB_DIM_DONE, Trigger.NONE)
    st.next_uop = (0, 1, 0)
    return [seed, lat, st]


class _Hinge2xOp:
    """Duck-type of dve_ops.DveOp with hand-written uops (+2x variant).

    Semantics per instruction (subdim form):
      in0 [P, S, V] bf16 (u block), in1 [P, 2S] bf16 (duplicated t'),
      out [P, S*V] bf16; out[:, -1] = sum_{s,n} relu(in0[p,s,n] + t'[p,s] + 1).
    """

    name = "HINGE2X_ANT"
    subdim = True

    def __init__(self):
        from concourse.dve_spec import Spec, Src0, Src1, relu

        def _ref(in0, in1, c0, c1, c2):
            a = np.asarray(in0, np.float32)
            S = a.shape[1]
            thr = np.asarray(in1, np.float32)[:, : 2 * S : 2]
            z = np.maximum(a + thr[:, :, None] + 1.0, 0.0)
            acc = z.sum((1, 2))
            zf = z.reshape(z.shape[0], -1).copy()
            zf[:, -1] = acc
            return zf

        self.spec = Spec(body=relu(Src0 + Src1), reference=_ref)
        self._cache = {}

    def compile(self, ver):
        from concourse.dve_uop import DveOpSpec
        from concourse.dve_ops import get_dve_sub_opcode

        if ver not in self._cache:
            r = DveOpSpec(
                name=self.name,
                opcode=get_dve_sub_opcode(self.name),
                uops=_build_hinge_uops(False),
                uops_2x=_build_hinge_uops(True),
                perf_max=1,
                rd1_en=True,
            )
            r.validate(ver)
            self._cache[ver] = r
        return self._cache[ver]


def _register_op():
    from concourse import dve_ops as dops

    if hasattr(dops, "ANT_HINGE2X_OP"):
        return dops.ANT_HINGE2X_OP
    op = _Hinge2xOp()
    opc = max(dops._SUB_OPCODE_FOR_NAME.values()) + 1
    dops.OPS.append(op)
    dops.CUSTOM_DVE_SPECS[op.name] = op.spec
    dops._SUB_OPCODE_FOR_NAME[op.name] = opc
    dops.ANT_HINGE2X_OP = op
    return op


HINGE2X = _register_op()


def _act_set_id(nc, name):
    from concourse.hw_specs import get_activation_tables

    return list(get_activation_tables(nc.m.arch)).index(name)


# --------------------------------------------------------------------------
# per-core bass program
# --------------------------------------------------------------------------

def build_nc(schedule):
    """schedule: per-block hinge-slot counts in COMPUTE order (descending)."""
    nc = bacc.Bacc("TRN2", target_bir_lowering=False, debug=False)
    xg_dram = nc.dram_tensor("xg", [P, XCOLS], BF16, kind="ExternalInput")
    out_dram = nc.dram_tensor("out", [P, NBLK + 1], F32, kind="ExternalOutput")
    xg_ap = xg_dram.ap()

    # input DMA split: chunk boundaries in xall columns
    CUT1 = T1X                 # dup table + two heaviest blocks
    CUT2 = U2 + 4 * V          # plain table + blocks 2..5

    with tile.TileContext(nc) as tc:
        with (
            tc.tile_pool(name="const", bufs=1) as cpool,
            tc.tile_pool(name="work", bufs=1) as wpool,
        ):
            nc.scalar.add_instruction(
                mybir.InstLoadActFuncSet(
                    name=nc.get_next_instruction_name(), ins=[], outs=[],
                    act_func_set_id=_act_set_id(
                        nc, "natural_log_exp_and_others"
                    ),
                )
            )
            xall = cpool.tile([P, XCOLS], BF16, tag="xall")
            nc.sync.dma_start(xall[:, 0:CUT1], xg_ap[:, 0:CUT1])
            nc.gpsimd.dma_start(xall[:, CUT1:CUT2], xg_ap[:, CUT1:CUT2])
            nc.sync.dma_start(xall[:, CUT2:XCOLS], xg_ap[:, CUT2:XCOLS])

            zarena = wpool.tile([P, NBLK * ZSLOT], BF16, tag="zarena")
            earena = wpool.tile([P, D_1X + NBLK * V], F32, tag="earena")
            acc = cpool.tile([P, NBLK + 1], F32, tag="acc")

            # ---- hinge: one custom DVE instruction per block ----
            for j in range(NBLK):
                S = schedule[j]
                x_blk = xall[:, _ucol(j) : _ucol(j) + V]
                # in0 reads the block's V columns S times (stride-0 repeat)
                in0 = x_blk.unsqueeze(1).broadcast_to([P, S, V])
                t2 = xall[:, j * 2 * SLOTS : j * 2 * SLOTS + 2 * S]
                z0 = (j + 1) * ZSLOT - S * V
                z = zarena[:, z0 : (j + 1) * ZSLOT]
                inst = nc.vector._custom_dve(
                    HINGE2X,
                    out=z.rearrange("p (s n) -> p s n", s=S),
                    in0=in0,
                    in1=t2,
                    s0=0.0, s1=0.0,
                )
                inst.ins.perf_max = 1

            # ---- BCE: exp then ln(1+e) with accum over [u0 | tb_1x | u1..] ----
            e_cuts = [D_DUP, CUT1, CUT2, XCOLS]
            for a, b in zip(e_cuts[:-1], e_cuts[1:]):
                nc.scalar.activation(
                    earena[:, a - D_DUP : b - D_DUP], xall[:, a:b],
                    ACTF.Exp, bias=0.0, scale=1.0,
                )
            lns = wpool.tile([P, D_1X + NBLK * V], F32, tag="lns")
            nc.scalar.activation(
                lns[:], earena[:], ACTF.Ln, bias=1.0, scale=1.0,
                accum_out=acc[:, NBLK : NBLK + 1],
            )

            # ---- gather the 16 block sums (last element of each z slot) ----
            gap = zarena[:].rearrange("p (b z) -> p b z", b=NBLK)[
                :, :, ZSLOT - 1 : ZSLOT
            ]
            nc.scalar.activation(
                acc[:, 0:NBLK], gap.squeeze(2), ACTF.Identity,
                bias=0.0, scale=1.0,
            )

            nc.sync.dma_start(out_dram.ap()[:, :], acc[:])

    nc.compile()
    return nc


_NC_CACHE = {}


def _get_nc(schedule):
    if schedule not in _NC_CACHE:
        _NC_CACHE[schedule] = build_nc(schedule)
    return _NC_CACHE[schedule]


def _pack_blocks(a, order):
    """[RPC, W] row-major -> [P, NBLK*W], blocks side by side in `order`."""
    w = a.shape[1]
    return np.ascontiguousarray(
        a.reshape(NBLK, P, w)[order].transpose(1, 0, 2).reshape(P, NBLK * w)
    )


def _shard(x, t):
    """npos-sorted round-robin shard, heaviest block first.

    Returns (schedule, in_maps); schedule is in compute (descending) order.
    """
    npos = (t > 0.5).sum(axis=1)
    assert npos.max() <= SLOTS, f"row with {npos.max()} positives > {SLOTS}"
    order = np.argsort(npos, kind="stable")
    ns = npos[order]
    sched_asc = [
        max(1, int(ns[(b + 1) * (N_CORES * P) - 1])) for b in range(NBLK)
    ]
    blk_order = list(range(NBLK - 1, -1, -1))     # descending slot counts
    schedule = tuple(sched_asc[b] for b in blk_order)
    xs = x[order]
    ps = t[order] > 0.5
    nsr = ns
    in_maps = []
    for c in range(N_CORES):
        xc = xs[c::N_CORES]                       # [RPC, V]
        pc = ps[c::N_CORES]
        nc_ = nsr[c::N_CORES]
        uc = np.where(pc, np.float32(NEGV), xc).astype(ml_dtypes.bfloat16)
        colorder = np.argsort(~pc, axis=1, kind="stable")[:, :SLOTS]
        vals = np.take_along_axis(xc, colorder, axis=1)
        mask = np.arange(SLOTS)[None, :] < nc_[:, None]
        tbl = np.where(mask, -vals, np.float32(PADV)).astype(ml_dtypes.bfloat16)
        tbl_dup = np.repeat(tbl, 2, axis=1)       # [RPC, 32]
        u_pack = _pack_blocks(uc, blk_order)
        in_maps.append(
            {
                "xg": np.concatenate(
                    [
                        _pack_blocks(tbl_dup, blk_order),
                        u_pack[:, 0 : 2 * V],
                        _pack_blocks(tbl, blk_order),
                        u_pack[:, 2 * V :],
                    ],
                    axis=1,
                )
            }
        )
    return schedule, in_maps


def _reduce_core(out_arr):
    """Device partials [P, NBLK+1] -> unnormalized core total
    V * sum_rows(0.7*bce_row + 0.3*mlm_row)."""
    a = out_arr.astype(np.float64)
    hinge = a[:, 0:NBLK].sum()
    sp = a[:, NBLK].sum()
    return BCE_W * sp + MLM_W * hinge


def kernel(logits: np.ndarray, targets: np.ndarray) -> np.ndarray:
    x = np.asarray(logits, dtype=np.float32).reshape(ROWS, V)
    t = np.asarray(targets, dtype=np.float32).reshape(ROWS, V)
    schedule, in_maps = _shard(x, t)
    nc = _get_nc(schedule)
    res = run_bass_kernel_spmd(nc, in_maps, list(range(N_CORES)))
    total = sum(
        _reduce_core(res.results[c]["out"]) for c in range(N_CORES)
    )
    return np.float32(total / V / ROWS)


# revision 13
# speedup vs baseline: 1.0285x; 1.0071x over previous
"""Trainium2 Bass kernel for 0.7*BCEWithLogits + 0.3*MultiLabelMarginLoss.

Math (per row of N = B*T rows, V = 128 classes; output = mean over rows):
  bce_row = mean_n[ softplus(x_n) - x_n*t_n ]
          = (1/V) [ sum_{neg} softplus(x_n) + sum_{pos} softplus(-x_p) ]
  mlm_row = (1/V) sum_{p in pos} sum_{n in neg} relu(1 - x_p + x_n)

Sharding: host sorts rows by positive count, deals them round-robin to the
8 cores (identical schedule per core -> one NEFF), packs each core's 16
row-blocks side by side as u [128, 16*128] bf16 with the positives masked
to -30, plus two threshold tables holding the NEGATED positive logits
t' = -x_p (pads -1e9): tb_dup [128, 16*32] with every slot value twice
(for the DVE hinge op's 2x word-latch), and tb_1x [128, 16*16] (for the
ACT softplus pass).

Device math per core:
  BCE:   one Exp pass + one Ln(1+e) pass with accum over [tb_1x | u]
         (softplus(t') = softplus(-x_p); pads/masks underflow to 0).
  Hinge: one custom DVE instruction per block running a hand-written
         3-state uop program (seed / latch / steady) in 2X_1PORT perf
         mode: the per-subdim threshold t' is latched from src1 (+1.0
         folded in via the ONE_F32 input lane), and the steady state
         computes relu(u + t' + 1) for TWO bf16 elements per cycle,
         keeping a running sum in block 7's out-flop which it streams to
         the dst; the LAST dst element of each block is the block's
         hinge total (this also holds if the engine falls back to 1x).
         Block z-slots are end-aligned at uniform stride in one arena so
         a single strided ACT copy gathers the 16 block sums into the
         accumulator tile.

Blocks are processed heaviest-first so the tail of the DVE stream is
cheap; inputs ship as 3 parallel DMAs (2 on the SP queue, 1 on the ACT
queue) so descriptor generation overlaps. The device ships acc [128, 17]
(16 hinge sums + 1 softplus sum); the host applies the 0.7/0.3 weights
and the mean. All arithmetic is on device; the host only permutes /
gathers / constant-fills input values and sums the 8 core partials.
"""

import sys

sys.path.insert(0, "/opt/trn_rl_repo")

import numpy as np
import ml_dtypes

import concourse.bacc as bacc
import concourse.tile as tile
from concourse import mybir
from concourse.bass_utils import run_bass_kernel_spmd

F32 = mybir.dt.float32
BF16 = mybir.dt.bfloat16
ACTF = mybir.ActivationFunctionType

B, T, V = 16, 1024, 128
ROWS = B * T
N_CORES = 8
RPC = ROWS // N_CORES             # 2048 rows per core
P = 128                           # rows per block
NBLK = RPC // P                   # 16 blocks
SLOTS = 16                        # positive-table slots per block
SMAX = 16                         # z-arena slot size in subdims (>= max npos)
ZSLOT = SMAX * V                  # z-arena columns per block

D_DUP = NBLK * 2 * SLOTS          # 512: duplicated threshold table cols
D_1X = NBLK * SLOTS               # 256: plain threshold table cols
XCOLS = D_DUP + D_1X + NBLK * V   # 2816 total input cols (bf16)
# xall layout: [ tb_dup | u_blk0 | u_blk1 | tb_1x | u_blk2..15 ] so the
# first DMA chunk carries exactly what the two heaviest hinges need.
U0A = D_DUP                       # u blocks 0-1 start
T1X = D_DUP + 2 * V               # plain table start
U2 = T1X + D_1X                   # u blocks 2.. start


def _ucol(j):
    return U0A + j * V if j < 2 else U2 + (j - 2) * V

PADV = -1.0e9                     # table pad (relu and softplus neutral)
NEGV = -30.0                      # masked-positive value in u
BCE_W = 0.7
MLM_W = 0.3


# --------------------------------------------------------------------------
# custom DVE op: 3-state subdim hinge with hand-written 1x and 2x programs
# --------------------------------------------------------------------------

def _build_hinge_uops(two_x: bool):
    """seed / latch / steady.

    Input lanes (chains): d0=SRC_0, d1=SRC_1, d2=ZERO, d3=SRC_0_HI (2x),
    d4=ONE_F32 (latch only; steady reuses d4/d5 internally).
    The latch computes t'+1 and captures it in blk1's swap flop (same-cycle
    combinational path), consuming one src1 element (1x, repeat=2 over the
    duplicated table) or one 2-element word (2x, repeat=1).
    The steady keeps the running sum in blk7's out-flop and streams it to
    the dst (WR0_LO at 1x, WR0_HI at 2x), so the last dst element is the
    block total.
    """
    from concourse.dve_uop import (
        AluInp, AluOp, DelayInp, InpSel, OutPath, OutSel, Trigger, UopConfig,
    )

    D = DelayInp
    A = AluInp

    def new_state(latch_lane=False):
        u = UopConfig()
        u.enable_input(InpSel.SRC_0, 1)       # chain d0
        u.enable_input(InpSel.SRC_1, 2)       # chain d1
        u.enable_input(InpSel.ZERO, 3)        # chain d2
        if two_x:
            u.enable_input(InpSel.SRC_0_HI, 4)  # chain d3
        if latch_lane:
            u.enable_input(InpSel.ONE_F32, 5)   # chain d4
        return u

    chains = (0, 1, 2, 3) if two_x else (0, 1, 2)

    # ---- state 0: SEED -- zero blk7's out-flop (the accumulator) ----
    seed = new_state()
    for b in range(7):
        seed.datapath_config[b].pass_through_delay(*chains)
    seed.datapath_config[7].enable_alu(AluOp.BYPASS, A.PREV_DELAY_2, A.PREV_DELAY_2)
    seed.repeat_count = 1
    seed.trigger = (Trigger.COUNT, Trigger.NONE, Trigger.NONE)
    seed.next_uop = (1, 0, 0)

    # ---- state 1: LATCH / STEP ----
    if two_x:
        # Consuming STEP (stock paged-mask pattern): one cycle at each
        # subdim boundary that consumes a src0 word AND a src1 word,
        # computes the body for that word with the fresh threshold routed
        # combinationally (blk0 -> blk1), and captures t'+1 in blk1's swap
        # for the following steady cycles. No stream bubble.
        lat = new_state(latch_lane=True)
        dp = lat.datapath_config
        dp[0].enable_alu(AluOp.ADD, A.PREV_DELAY_1, A.PREV_DELAY_4)  # t'+1
        dp[0].pass_through_delay(0, 2, 3)
        b1 = dp[1]
        b1.enable_alu(AluOp.ADD, A.PREV_DELAY_0, A.PREV_ALU_OUT)     # x0
        b1.swap_enable = 1                    # captures operand b = t'+1
        b1.pass_through_delay(0, 2, 3)
        dp[2].enable_alu(AluOp.MAX, A.PREV_ALU_OUT, A.PREV_DELAY_2)  # relu0
        dp[2].enable_delay_from_src(D.PREV_ALU_OUT, 4)               # d4 <- x0
        dp[2].pass_through_delay(0, 3)
        dp[3].enable_alu(AluOp.ADD, A.PREV_ALU_OUT, A.PREV_DELAY_4)  # relu0+x0
        dp[3].enable_delay_from_src(D.PREV_ALU_OUT, 5)               # d5 <- relu0
        dp[3].pass_through_delay(0, 3)
        dp[4].enable_alu(AluOp.ADD, A.PREV_ALU_OUT, A.PREV_DELAY_3)  # + S0H
        dp[4].pass_through_delay(0, 5)
        dp[5].enable_alu(AluOp.SUBTRACT, A.PREV_ALU_OUT, A.PREV_DELAY_0)
        dp[5].pass_through_delay(5)
        dp[6].enable_alu(AluOp.MAX, A.PREV_ALU_OUT, A.PREV_DELAY_5)  # pairsum
        dp[6].pass_through_delay(5)
        dp[7].enable_alu(AluOp.ADD, A.CURR_ALU_OUT, A.PREV_ALU_OUT)  # acc
        dp[7].pass_through_delay(5)
        lat.enable_output(OutSel.DELAY_5, OutPath.WR0_LO)
        lat.enable_output(OutSel.ALU_OUT, OutPath.WR0_HI)
        lat.require_inp0 = 1
        lat.require_inp1 = 1
        lat.repeat_count = 1
        lat.trigger = (Trigger.SRC_TENSOR_DONE, Trigger.SUB_DIM_DONE,
                       Trigger.COUNT)
        lat.next_uop = (0, 1, 2)
    else:
        # Non-consuming latch: swap(blk1) <- t' + 1, consuming 2 elements
        # of the duplicated table (repeat=2).
        lat = new_state(latch_lane=True)
        lc = chains + (4,)
        b0 = lat.datapath_config[0]
        b0.enable_alu(AluOp.ADD, A.PREV_DELAY_1, A.PREV_DELAY_4)   # t' + 1
        b0.pass_through_delay(*lc)
        b1 = lat.datapath_config[1]
        b1.op = AluOp.BYPASS
        b1.alu_src0 = A.PREV_ALU_OUT
        b1.alu_src1 = A.PREV_ALU_OUT
        b1.swap_enable = 1                                         # capture
        b1.pass_through_delay(*lc)
        for b in range(2, 7):
            lat.datapath_config[b].pass_through_delay(*lc)
        # blk7 untouched: preserves the running sum across subdims
        lat.require_inp1 = 1
        lat.repeat_count = 2
        lat.trigger = (Trigger.COUNT, Trigger.NONE, Trigger.NONE)
        lat.next_uop = (2, 0, 0)

    # ---- state 2: STEADY ----
    st = new_state()
    st.require_inp0 = 1
    dp = st.datapath_config
    if two_x:
        # x0 = S0 + (t'+1); relu0 = max(x0, 0)
        # x1 + relu0 = relu0 + x0 + S0H - S0; pairsum = max(x1+relu0, relu0)
        dp[0].pass_through_delay(0, 1, 2, 3)
        dp[1].enable_alu(AluOp.ADD, A.PREV_DELAY_0, A.CURR_SWAP_OUT)
        dp[1].pass_through_delay(0, 2, 3)
        dp[2].enable_alu(AluOp.MAX, A.PREV_ALU_OUT, A.PREV_DELAY_2)
        dp[2].enable_delay_from_src(D.PREV_ALU_OUT, 4)         # d4 <- x0
        dp[2].pass_through_delay(0, 3)
        dp[3].enable_alu(AluOp.ADD, A.PREV_ALU_OUT, A.PREV_DELAY_4)  # relu0+x0
        dp[3].enable_delay_from_src(D.PREV_ALU_OUT, 5)         # d5 <- relu0
        dp[3].pass_through_delay(0, 3)
        dp[4].enable_alu(AluOp.ADD, A.PREV_ALU_OUT, A.PREV_DELAY_3)  # + S0H
        dp[4].pass_through_delay(0, 5)
        dp[5].enable_alu(AluOp.SUBTRACT, A.PREV_ALU_OUT, A.PREV_DELAY_0)  # - S0
        dp[5].pass_through_delay(5)
        dp[6].enable_alu(AluOp.MAX, A.PREV_ALU_OUT, A.PREV_DELAY_5)  # pairsum
        dp[6].pass_through_delay(5)
        dp[7].enable_alu(AluOp.ADD, A.CURR_ALU_OUT, A.PREV_ALU_OUT)  # acc
        dp[7].pass_through_delay(5)
        st.enable_output(OutSel.DELAY_5, OutPath.WR0_LO)       # relu0
        st.enable_output(OutSel.ALU_OUT, OutPath.WR0_HI)       # running acc
    else:
        dp[0].pass_through_delay(0, 1, 2)
        dp[1].enable_alu(AluOp.ADD, A.PREV_DELAY_0, A.CURR_SWAP_OUT)
        dp[1].pass_through_delay(2)
        dp[2].enable_alu(AluOp.MAX, A.PREV_ALU_OUT, A.PREV_DELAY_2)  # relu
        for b in (3, 4, 5, 6):
            dp[b].enable_alu(AluOp.BYPASS, A.PREV_ALU_OUT, A.PREV_ALU_OUT)
        dp[7].enable_alu(AluOp.ADD, A.CURR_ALU_OUT, A.PREV_ALU_OUT)  # acc
        st.enable_output(OutSel.ALU_OUT, OutPath.WR0_LO)       # running acc
    st.trigger = (Trigger.SRC_TENSOR_DONE, Trigger.SUB_DIM_DONE, Trigger.NONE)
    st.next_uop = (0, 1, 0)
    return [seed, lat, st]


class _Hinge2xOp:
    """Duck-type of dve_ops.DveOp with hand-written uops (+2x variant).

    Semantics per instruction (subdim form):
      in0 [P, S, V] bf16 (u block), in1 [P, 2S] bf16 (duplicated t'),
      out [P, S*V] bf16; out[:, -1] = sum_{s,n} relu(in0[p,s,n] + t'[p,s] + 1).
    """

    name = "HINGE2X_ANT"
    subdim = True

    def __init__(self):
        from concourse.dve_spec import Spec, Src0, Src1, relu

        def _ref(in0, in1, c0, c1, c2):
            a = np.asarray(in0, np.float32)
            S = a.shape[1]
            thr = np.asarray(in1, np.float32)[:, : 2 * S : 2]
            z = np.maximum(a + thr[:, :, None] + 1.0, 0.0)
            acc = z.sum((1, 2))
            zf = z.reshape(z.shape[0], -1).copy()
            zf[:, -1] = acc
            return zf

        self.spec = Spec(body=relu(Src0 + Src1), reference=_ref)
        self._cache = {}

    def compile(self, ver):
        from concourse.dve_uop import DveOpSpec
        from concourse.dve_ops import get_dve_sub_opcode

        if ver not in self._cache:
            r = DveOpSpec(
                name=self.name,
                opcode=get_dve_sub_opcode(self.name),
                uops=_build_hinge_uops(False),
                uops_2x=_build_hinge_uops(True),
                perf_max=1,
                rd1_en=True,
            )
            r.validate(ver)
            self._cache[ver] = r
        return self._cache[ver]


def _register_op():
    from concourse import dve_ops as dops

    if hasattr(dops, "ANT_HINGE2X_OP"):
        return dops.ANT_HINGE2X_OP
    op = _Hinge2xOp()
    opc = max(dops._SUB_OPCODE_FOR_NAME.values()) + 1
    dops.OPS.append(op)
    dops.CUSTOM_DVE_SPECS[op.name] = op.spec
    dops._SUB_OPCODE_FOR_NAME[op.name] = opc
    dops.ANT_HINGE2X_OP = op
    return op


HINGE2X = _register_op()


def _act_set_id(nc, name):
    from concourse.hw_specs import get_activation_tables

    return list(get_activation_tables(nc.m.arch)).index(name)


# --------------------------------------------------------------------------
# per-core bass program
# --------------------------------------------------------------------------

def build_nc(schedule):
    """schedule: per-block hinge-slot counts in COMPUTE order (descending)."""
    nc = bacc.Bacc("TRN2", target_bir_lowering=False, debug=False)
    xg_dram = nc.dram_tensor("xg", [P, XCOLS], BF16, kind="ExternalInput")
    out_dram = nc.dram_tensor("out", [P, NBLK + 1], F32, kind="ExternalOutput")
    xg_ap = xg_dram.ap()

    # input DMA split: chunk boundaries in xall columns
    CUT1 = T1X                 # dup table + two heaviest blocks
    CUT2 = U2 + 4 * V          # plain table + blocks 2..5

    with tile.TileContext(nc) as tc:
        with (
            tc.tile_pool(name="const", bufs=1) as cpool,
            tc.tile_pool(name="work", bufs=1) as wpool,
        ):
            nc.scalar.add_instruction(
                mybir.InstLoadActFuncSet(
                    name=nc.get_next_instruction_name(), ins=[], outs=[],
                    act_func_set_id=_act_set_id(
                        nc, "natural_log_exp_and_others"
                    ),
                )
            )
            xall = cpool.tile([P, XCOLS], BF16, tag="xall")
            nc.sync.dma_start(xall[:, 0:CUT1], xg_ap[:, 0:CUT1])
            nc.gpsimd.dma_start(xall[:, CUT1:CUT2], xg_ap[:, CUT1:CUT2])
            nc.sync.dma_start(xall[:, CUT2:XCOLS], xg_ap[:, CUT2:XCOLS])

            zarena = wpool.tile([P, NBLK * ZSLOT], BF16, tag="zarena")
            earena = wpool.tile([P, D_1X + NBLK * V], F32, tag="earena")
            acc = cpool.tile([P, NBLK + 1], F32, tag="acc")

            # ---- hinge: one custom DVE instruction per block ----
            for j in range(NBLK):
                S = schedule[j]
                x_blk = xall[:, _ucol(j) : _ucol(j) + V]
                # in0 reads the block's V columns S times (stride-0 repeat)
                in0 = x_blk.unsqueeze(1).broadcast_to([P, S, V])
                t2 = xall[:, j * 2 * SLOTS : j * 2 * SLOTS + 2 * S]
                z0 = (j + 1) * ZSLOT - S * V
                z = zarena[:, z0 : (j + 1) * ZSLOT]
                inst = nc.vector._custom_dve(
                    HINGE2X,
                    out=z.rearrange("p (s n) -> p s n", s=S),
                    in0=in0,
                    in1=t2,
                    s0=0.0, s1=0.0,
                )
                inst.ins.perf_max = 1

            # ---- BCE: exp then ln(1+e) with accum over [u0 | tb_1x | u1..] ----
            e_cuts = [D_DUP, CUT1, CUT2, XCOLS]
            for a, b in zip(e_cuts[:-1], e_cuts[1:]):
                nc.scalar.activation(
                    earena[:, a - D_DUP : b - D_DUP], xall[:, a:b],
                    ACTF.Exp, bias=0.0, scale=1.0,
                )
            lns = wpool.tile([P, D_1X + NBLK * V], F32, tag="lns")
            nc.scalar.activation(
                lns[:], earena[:], ACTF.Ln, bias=1.0, scale=1.0,
                accum_out=acc[:, NBLK : NBLK + 1],
            )

            # ---- gather the 16 block sums (last element of each z slot) ----
            gap = zarena[:].rearrange("p (b z) -> p b z", b=NBLK)[
                :, :, ZSLOT - 1 : ZSLOT
            ]
            nc.scalar.activation(
                acc[:, 0:NBLK], gap.squeeze(2), ACTF.Identity,
                bias=0.0, scale=1.0,
            )

            nc.sync.dma_start(out_dram.ap()[:, :], acc[:])

    nc.compile()
    return nc


_NC_CACHE = {}


def _get_nc(schedule):
    if schedule not in _NC_CACHE:
        _NC_CACHE[schedule] = build_nc(schedule)
    return _NC_CACHE[schedule]


def _pack_blocks(a, order):
    """[RPC, W] row-major -> [P, NBLK*W], blocks side by side in `order`."""
    w = a.shape[1]
    return np.ascontiguousarray(
        a.reshape(NBLK, P, w)[order].transpose(1, 0, 2).reshape(P, NBLK * w)
    )


def _shard(x, t):
    """npos-sorted round-robin shard, heaviest block first.

    Returns (schedule, in_maps); schedule is in compute (descending) order.
    """
    npos = (t > 0.5).sum(axis=1)
    assert npos.max() <= SLOTS, f"row with {npos.max()} positives > {SLOTS}"
    order = np.argsort(npos, kind="stable")
    ns = npos[order]
    sched_asc = [
        max(1, int(ns[(b + 1) * (N_CORES * P) - 1])) for b in range(NBLK)
    ]
    blk_order = list(range(NBLK - 1, -1, -1))     # descending slot counts
    schedule = tuple(sched_asc[b] for b in blk_order)
    xs = x[order]
    ps = t[order] > 0.5
    nsr = ns
    in_maps = []
    for c in range(N_CORES):
        xc = xs[c::N_CORES]                       # [RPC, V]
        pc = ps[c::N_CORES]
        nc_ = nsr[c::N_CORES]
        uc = np.where(pc, np.float32(NEGV), xc).astype(ml_dtypes.bfloat16)
        colorder = np.argsort(~pc, axis=1, kind="stable")[:, :SLOTS]
        vals = np.take_along_axis(xc, colorder, axis=1)
        mask = np.arange(SLOTS)[None, :] < nc_[:, None]
        tbl = np.where(mask, -vals, np.float32(PADV)).astype(ml_dtypes.bfloat16)
        tbl_dup = np.repeat(tbl, 2, axis=1)       # [RPC, 32]
        u_pack = _pack_blocks(uc, blk_order)
        in_maps.append(
            {
                "xg": np.concatenate(
                    [
                        _pack_blocks(tbl_dup, blk_order),
                        u_pack[:, 0 : 2 * V],
                        _pack_blocks(tbl, blk_order),
                        u_pack[:, 2 * V :],
                    ],
                    axis=1,
                )
            }
        )
    return schedule, in_maps


def _reduce_core(out_arr):
    """Device partials [P, NBLK+1] -> unnormalized core total
    V * sum_rows(0.7*bce_row + 0.3*mlm_row)."""
    a = out_arr.astype(np.float64)
    hinge = a[:, 0:NBLK].sum()
    sp = a[:, NBLK].sum()
    return BCE_W * sp + MLM_W * hinge


def kernel(logits: np.ndarray, targets: np.ndarray) -> np.ndarray:
    x = np.asarray(logits, dtype=np.float32).reshape(ROWS, V)
    t = np.asarray(targets, dtype=np.float32).reshape(ROWS, V)
    schedule, in_maps = _shard(x, t)
    nc = _get_nc(schedule)
    res = run_bass_kernel_spmd(nc, in_maps, list(range(N_CORES)))
    total = sum(
        _reduce_core(res.results[c]["out"]) for c in range(N_CORES)
    )
    return np.float32(total / V / ROWS)
